# revision 26
# baseline (speedup 1.0000x reference)
"""MoE layer (E=8 experts, top-2, swiglu FFN) on 8 Trainium2 NeuronCores.

Strategy: expert dispatch on host + tensor-parallel-over-hidden on device.
  - Router (logits -> top-2 -> softmax weights) computed on host with the
    exact same jnp ops as the reference, so top-k decisions match bit-for-bit.
  - Tokens are gathered per expert on host into one flat dispatch list
    shared by all cores.
  - Every core processes ALL experts' token lists, but only a 1/8 slice of
    each expert's hidden units (h1 cols [256c:256c+256) paired with the
    matching h2 cols, and the matching W2 rows). The hidden split commutes
    with swiglu, so partial outputs sum exactly. Per-core work is exactly
    sum(n_e)/8 token-equivalents — perfectly balanced, no padding (the
    matmul moving dim takes arbitrary tile sizes).
  - On-device dataflow: features on partitions / tokens on the moving free
    dim; H^T = W1ᵀ·X^T, swiglu, Y^T = W2ᵀ·S^T — no on-chip transposes.
    bf16 matmuls, fp32 accumulate, fp16 partial-y output.
  - Host combines: out[token] += w_k * (sum_c y_c + b2[e]).
"""

import numpy as np
import ml_dtypes

E = 8
K = 2
DIM = 1024
HID = 2048
H2 = 2 * HID  # fc1 output width (4096)
P = 128
KO1 = DIM // P  # 8 k-tiles for fc1
SH = HID // 8  # 256 hidden units per core-shard
SW = 2 * SH  # 512 fc1 output cols per shard (h1 half + h2 half)
MO1 = SW // P  # 4 m-tiles for fc1 shard output (0,1 = h1; 2,3 = h2)
KO2 = SH // P  # 2 k-tiles for fc2 shard
MO2 = DIM // P  # 8 m-tiles for fc2 output
TN = 512  # max token tile (matmul moving free dim)

_cache: dict = {}

# Extra kwargs splatted into run_bass_kernel_spmd (test harness sets this to
# enable NTFF tracing; empty by default so grading runs are unaffected).
TRACE_OPTS: dict = {}
LAST_RESULTS = None


def _tile_list(counts):
    """[(expert, tn), ...] covering each expert's token list.

    Per-expert BALANCED splits (all tiles ~ne/ceil(ne/TN)) instead of
    512+remainder: a tiny remainder tile (e.g. 2 tokens) costs 48 matmuls
    of pure ~25ns dispatch overhead (~2.4us for two such tiles), while
    balanced tiles keep every matmul's column count large enough to hide
    dispatch under execute. Experts ordered by descending min tile size
    (so the first tile is the biggest, for the cold-start load) and sizes
    descend within an expert. The globally-last expert donates a ~160-col
    final tile so the end-of-kernel store drain is short.
    """
    E_ = len(counts)
    splits = {}
    for e in range(E_):
        ne = counts[e]
        if ne == 0:
            splits[e] = []
            continue
        nt = -(-ne // TN)
        base, ext = divmod(ne, nt)
        splits[e] = [base + 1] * ext + [base] * (nt - ext)
    order = sorted((e for e in range(E_) if splits[e]), key=lambda e: -min(splits[e]))
    if order:
        last = order[-1]
        ne, nt = counts[last], len(splits[last])
        small = 160
        if nt > 1 and ne > 2 * small and ne - small <= (nt - 1) * TN:
            base, ext = divmod(ne - small, nt - 1)
            splits[last] = [base + 1] * ext + [base] * (nt - 1 - ext) + [small]
    return [(e, tn) for e in order for tn in splits[e]]


def _build(counts):
    """Build + compile the SPMD Bass program for per-expert token counts."""
    import concourse.mybir as mybir
    import concourse.tile as tile
    from concourse import bacc
    from contextlib import ExitStack

    dt = mybir.dt
    AF = mybir.ActivationFunctionType
    ALU = mybir.AluOpType

    tiles = _tile_list(counts)
    ntiles = len(tiles)

    nc = bacc.Bacc("TRN2", target_bir_lowering=False, debug=False, num_devices=8)

    xt = nc.dram_tensor(
        "xt", [ntiles, P, KO1 * TN], dt.bfloat16, kind="ExternalInput"
    ).ap()
    w1 = nc.dram_tensor("w1", [E, P, KO1 * SW], dt.bfloat16, kind="ExternalInput").ap()
    b1 = nc.dram_tensor("b1", [P, E * MO1], dt.float32, kind="ExternalInput").ap()
    w2 = nc.dram_tensor("w2", [E, P, KO2 * DIM], dt.bfloat16, kind="ExternalInput").ap()
    yt = nc.dram_tensor(
        "yt", [ntiles, P, MO2 * TN], dt.float16, kind="ExternalOutput"
    ).ap()

    with tile.TileContext(nc) as tc, ExitStack() as ctx:
        wpool = ctx.enter_context(tc.tile_pool(name="weights", bufs=1))
        xpool = ctx.enter_context(tc.tile_pool(name="xp", bufs=5))
        spool = ctx.enter_context(tc.tile_pool(name="sp", bufs=2))
        opool = ctx.enter_context(tc.tile_pool(name="op", bufs=2))
        tpool = ctx.enter_context(tc.tile_pool(name="tp", bufs=3))
        pspool = ctx.enter_context(tc.tile_pool(name="ps", bufs=4, space="PSUM"))
        pypool = ctx.enter_context(tc.tile_pool(name="py", bufs=4, space="PSUM"))

        w1_sb = wpool.tile([P, E, KO1, SW], dt.bfloat16)
        w2_sb = wpool.tile([P, E, KO2, DIM], dt.bfloat16)
        b1_sb = wpool.tile([P, E * MO1], dt.float32)

        # PE warmup: junk matmuls on a zeroed tile bridge the queue preamble /
        # DMA spin-up window so the clock gate opens before real work. 19 of
        # them (~8us at the mid p-state's ~427ns each) cover the whole
        # un-ramped cold-DMA window for tile 0's 2MB: real matmuls then
        # start fully ramped with data resident. Shorter warmups leave a
        # >2.5us mid-stream DMA stall, which trips the p-state hysteresis
        # and costs ~10 extra half-speed matmuls on re-ramp.
        warm_sb = wpool.tile([P, TN], dt.bfloat16)
        nc.vector.memset(warm_sb[:], 0.0)
        warm_ps = pypool.tile([P, TN], dt.float32, tag="py")
        for _ in range(21):
            nc.tensor.matmul(
                warm_ps[:],
                lhsT=warm_sb[:, :P],
                rhs=warm_sb[:, :],
                start=True,
                stop=True,
            )

        # Cold-start plan: tile 0's fc1 consumes one k-slice at a time
        # (4 matmuls per k, see the k-outer branch below), so the first-use
        # rate is ~290GB/s — coverable by running BOTH HWDGE rings: the
        # fast sync ring q1 (~250GB/s cold) carries x0 (k-pairs) inter-
        # leaved with w1[e0]'s even k-slices, the slower scalar ring q10
        # (~100GB/s) carries the odd k-slices and then w2[e0]. Each piece's
        # ~700ns dma issue pipelines behind the previous piece's transfer.
        exp_order = list(dict.fromkeys(e for e, _ in tiles))
        e0, tn0 = tiles[0]
        # Tiles 0 and 1 (same expert) run a FUSED k-outer fc1 using all 8
        # PSUM banks, so each k-slice of w1[e0]/x0/x1 is first-used at only
        # ~217GB/s — robust against per-core cold-DMA ramp jitter (at
        # ~290GB/s single-tile demand, an unlucky core stalls and trips the
        # p-state hysteresis for several us).
        fused01 = ntiles >= 2 and tiles[1][0] == e0
        tn1 = tiles[1][1] if fused01 else 0
        x_first = xpool.tile([P, KO1, tn0], dt.bfloat16, tag="x")
        # b1 (16KB, first needed at the tile-0 epilogue ~20us in) rides the
        # otherwise-idle gpsimd queue so it doesn't delay x0 on the sync
        # ring during the un-ramped cold-start phase. Only sync (SP),
        # scalar (Activation) and gpsimd can issue DMAs on TRN2.
        nc.gpsimd.dma_start(b1_sb[:], b1[:])
        x0_flat = x_first[:].rearrange("p k n -> p (k n)")
        w1e0_flat = w1_sb[:, e0].rearrange("p k n -> p (k n)")
        if fused01:
            x_second = xpool.tile([P, KO1, tn1], dt.bfloat16, tag="x", name="x_1")
            x1_flat = x_second[:].rearrange("p k n -> p (k n)")
        for k in range(0, KO1, 2):
            nc.sync.dma_start(
                w1e0_flat[:, k * SW : (k + 1) * SW], w1[e0, :, k * SW : (k + 1) * SW]
            )
            nc.sync.dma_start(
                x0_flat[:, k * tn0 : (k + 2) * tn0],
                xt[0, :, k * tn0 : (k + 2) * tn0],
            )
            if fused01:
                nc.sync.dma_start(
                    x1_flat[:, k * tn1 : (k + 2) * tn1],
                    xt[1, :, k * tn1 : (k + 2) * tn1],
                )
        for k in range(1, KO1, 2):
            nc.scalar.dma_start(
                w1e0_flat[:, k * SW : (k + 1) * SW], w1[e0, :, k * SW : (k + 1) * SW]
            )
        # Pre-emit the whole DMA schedule in consumption order on the sync
        # FIFO: expert e's weights interleaved with the x tiles consumed
        # around the same time. Emitting x loads inside the tile loop would
        # queue them behind megabytes of weight transfers and starve the PE.
        x_tiles = {0: x_first}
        if fused01:
            x_tiles[1] = x_second

        def emit_x(ti, q=None):
            if ti >= ntiles or ti in x_tiles:
                return
            tn = tiles[ti][1]
            # xt is packed compactly per tile ([P, KO1*tn] used cols), so
            # every x load is one fully-contiguous transfer (4KB packets).
            xx = xpool.tile([P, KO1, tn], dt.bfloat16, tag="x", name=f"x_{ti}")
            (q or nc.sync).dma_start(
                xx[:].rearrange("p k n -> p (k n)"), xt[ti, :, : KO1 * tn]
            )
            x_tiles[ti] = xx

        def emit_w(e):
            nc.sync.dma_start(w1_sb[:, e].rearrange("p k n -> p (k n)"), w1[e])
            nc.sync.dma_start(w2_sb[:, e].rearrange("p k n -> p (k n)"), w2[e])

        # Per-core HBM throughput varies run to run (a degraded core's sync
        # ring sustains only ~190GB/s vs ~280 typical); front-loading all
        # 11MB of expert weights on q1 then starves the x prefetch ~10 tiles
        # in, a 6us PE stall that also trips the p-state hysteresis. So:
        # x2/x3 ride the otherwise-idle scalar ring, only the next 2
        # experts' weights + x4..x7 are pre-emitted on sync, and everything
        # later is issued from inside the tile loop (see below) where the
        # xpool buffer being recycled is free by construction, alternating
        # x tiles between the sync and scalar rings.
        nc.scalar.dma_start(w2_sb[:, e0].rearrange("p k n -> p (k n)"), w2[e0])
        emit_x(2, nc.scalar)
        emit_x(3)
        nxt = 4
        for e in exp_order[1:3]:
            emit_x(nxt)
            emit_x(nxt + 1)
            emit_w(e)
            nxt += 2
        # In-loop emission schedule: at tile ti, x_{ti+5} goes out right
        # after the epilogue (its pool buffer was freed by tile ti's own
        # fc1), and the weights of expert exp_order[j] go out when the
        # previous expert starts (one expert of lead time).
        late_x = {ti: ti + 5 for ti in range(nxt - 5, ntiles - 5)}
        late_w = {}
        first_tile = {}
        for ti, (e, _) in enumerate(tiles):
            first_tile.setdefault(e, ti)
        for j in range(3, len(exp_order)):
            late_w[first_tile[exp_order[j - 1]]] = exp_order[j]

        pend_ps = {}
        for ti, (e, tn) in enumerate(tiles):
            x_sb = x_tiles[ti]

            # Stage 1: 4 m-tiles (2 h1 + 2 h2), k-loop outermost so each
            # arriving DMA slice unblocks dense matmuls during the cold start.
            s_sb = spool.tile([P, KO2, TN], dt.bfloat16, tag="s")
            if ti in pend_ps:
                ps1s, ps2s = pend_ps.pop(ti)
            else:
                ps1s = [
                    pspool.tile([P, TN], dt.float32, tag="ps", name=f"ps1_{ti}_{i}")
                    for i in range(2)
                ]
                ps2s = [
                    pspool.tile([P, TN], dt.float32, tag="ps", name=f"ps2_{ti}_{i}")
                    for i in range(2)
                ]
            if ti == 0 and fused01:
                # Fused k-outer fc1 over tiles 0 and 1: 8 matmuls per k
                # across 8 live psum banks (tile 0 in pspool, tile 1 in
                # pypool), so each arriving k-slice is consumed once at the
                # lowest possible rate. Tile 0's psums close first within
                # the final k so its epilogue starts early; tile 1's fc1
                # results are stashed for its loop iteration.
                ps1b = [
                    pypool.tile([P, TN], dt.float32, tag="py", name=f"ps1b_{i}")
                    for i in range(2)
                ]
                ps2b = [
                    pypool.tile([P, TN], dt.float32, tag="py", name=f"ps2b_{i}")
                    for i in range(2)
                ]
                x1_sb = x_tiles[1]
                for k in range(KO1):
                    for mi in range(2):
                        for pss, mo in ((ps1s, 0), (ps2s, 2)):
                            nc.tensor.matmul(
                                pss[mi][:, :tn],
                                lhsT=w1_sb[:, e, k, (mo + mi) * P : (mo + mi + 1) * P],
                                rhs=x_sb[:, k, :tn],
                                start=(k == 0),
                                stop=(k == KO1 - 1),
                            )
                    for mi in range(2):
                        for pss, mo in ((ps1b, 0), (ps2b, 2)):
                            nc.tensor.matmul(
                                pss[mi][:, :tn1],
                                lhsT=w1_sb[:, e, k, (mo + mi) * P : (mo + mi + 1) * P],
                                rhs=x1_sb[:, k, :tn1],
                                start=(k == 0),
                                stop=(k == KO1 - 1),
                            )
                pend_ps[1] = (ps1b, ps2b)
            elif ti == 0:
                # Tile 0 runs fully k-outer (all 4 psum tiles per k): each
                # k-slice of x0/w1[e0] is touched exactly once, so the cold
                # DMA stream only has to keep up with ~290GB/s of FIRST use
                # instead of ps1's 2-pass ~570GB/s. mi=0 psums close first
                # within the final k so the epilogue starts 2 matmuls early.
                for k in range(KO1):
                    for mi in range(2):
                        for pss, mo in ((ps1s, 0), (ps2s, 2)):
                            nc.tensor.matmul(
                                pss[mi][:, :tn],
                                lhsT=w1_sb[:, e, k, (mo + mi) * P : (mo + mi + 1) * P],
                                rhs=x_sb[:, k, :tn],
                                start=(k == 0),
                                stop=(k == KO1 - 1),
                            )
            elif ti != 1 or not fused01:
                # fc1 in two half-k chunks so the tile's x is FIRST-used
                # over 32 matmuls (~148GB/s) instead of ps1's single pass
                # over 16 (~290GB/s) — on a core whose DMA ring is running
                # degraded (~190GB/s happens), the old order stalled the PE
                # right after the fused cold start. Within the second chunk
                # psums run mi-outer and ps1s[0] closes 12 matmuls early, so
                # the mi=0 epilogue still overlaps fc1's tail and fc2 starts
                # without a bubble.
                half = KO1 // 2
                for k in range(half):
                    for mi in range(2):
                        nc.tensor.matmul(
                            ps1s[mi][:, :tn],
                            lhsT=w1_sb[:, e, k, mi * P : (mi + 1) * P],
                            rhs=x_sb[:, k, :tn],
                            start=(k == 0),
                            stop=False,
                        )
                for k in range(half):
                    for mi in range(2):
                        nc.tensor.matmul(
                            ps2s[mi][:, :tn],
                            lhsT=w1_sb[:, e, k, (2 + mi) * P : (3 + mi) * P],
                            rhs=x_sb[:, k, :tn],
                            start=(k == 0),
                            stop=False,
                        )
                for pss, mo in ((ps1s, 0), (ps2s, 2)):
                    for mi in range(2):
                        for k in range(half, KO1):
                            nc.tensor.matmul(
                                pss[mi][:, :tn],
                                lhsT=w1_sb[:, e, k, (mo + mi) * P : (mo + mi + 1) * P],
                                rhs=x_sb[:, k, :tn],
                                start=False,
                                stop=(k == KO1 - 1),
                            )
            for mi in range(2):
                t1 = tpool.tile([P, TN], dt.float32, tag="t1")
                # t1 = silu(h1 + b1a)
                nc.scalar.activation(
                    t1[:, :tn],
                    ps1s[mi][:, :tn],
                    AF.Silu,
                    bias=b1_sb[:, e * MO1 + mi : e * MO1 + mi + 1],
                )
                # s = (h2 + b1b) * t1   (cast to bf16 on write)
                nc.vector.scalar_tensor_tensor(
                    s_sb[:, mi, :tn],
                    ps2s[mi][:, :tn],
                    b1_sb[:, e * MO1 + 2 + mi : e * MO1 + 3 + mi],
                    t1[:, :tn],
                    op0=ALU.add,
                    op1=ALU.mult,
                )

            # Deferred prefetch (see the emission-schedule comment above).
            # Odd x tiles ride the scalar ring: its dma issue slots here in
            # the scalar FIFO between the silus and the fc2 psum copies,
            # hiding entirely in scalar idle time.
            if ti in late_x:
                tx = late_x[ti]
                emit_x(tx, nc.scalar if tx % 2 else nc.sync)
            if ti in late_w:
                emit_w(late_w[ti])

            # Stage 2: partial y for this shard (no b2 — host adds it once).
            # All 8 m2 slices collect into one SBUF tile and leave as a
            # single store on the gpsimd queue: dma_start issue costs ~600ns
            # of engine time each, so per-m2 stores on the load FIFO would
            # head-of-line-block the x/weight stream.
            # o_sb is sized [P, MO2, tn] (not TN) and stored to yt[ti]'s
            # leading MO2*tn cols COMPACTLY: a remainder tile's strided
            # store otherwise degenerates to tn*2-byte DMA packets (4 bytes
            # for the tn=2 tile!), taking ~3us to drain at kernel end.
            o_sb = opool.tile([P, MO2, tn], dt.float16, tag="o")
            # The final tiles store via the fast HWDGE ring (low-latency
            # completion on the drain path); earlier tiles use the gpsimd
            # SWDGE queue so the stores don't head-of-line-block the
            # x/weight stream on q1. Each store goes out in two halves so
            # the first half streams while the second half's psum copies run.
            # fc2 in two m2-groups of 4, k2-phase-ordered: the group's k2=0
            # matmuls need only s_sb[:,0], so fc2 starts immediately after
            # fc1's last matmul while the mi=1 epilogue (s_sb[:,1]) finishes
            # on the side engines. 4 live psy tiles = pypool exactly.
            q = nc.sync if ti >= ntiles - 2 else nc.gpsimd
            # Tile 0's fc2 psums come from pspool (freed by its own
            # epilogue) when pypool is still holding tile 1's fused fc1.
            ppool, ptag = (pspool, "ps") if (ti == 0 and fused01) else (pypool, "py")
            for g in range(2):
                m2s = list(range(g * 4, g * 4 + 4))
                psys = [
                    ppool.tile([P, TN], dt.float32, tag=ptag, name=f"psy_{ti}_{m2}")
                    for m2 in m2s
                ]
                for k2 in range(KO2):
                    for j, m2 in enumerate(m2s):
                        nc.tensor.matmul(
                            psys[j][:, :tn],
                            lhsT=w2_sb[:, e, k2, m2 * P : (m2 + 1) * P],
                            rhs=s_sb[:, k2, :tn],
                            start=(k2 == 0),
                            stop=(k2 == KO2 - 1),
                        )
                # Alternate the psum->SBUF copies between ScalarE and VectorE:
                # a single engine can't keep up at this tile rate.
                for j, m2 in enumerate(m2s):
                    if m2 % 2 == 0:
                        nc.scalar.copy(o_sb[:, m2, :], psys[j][:, :tn])
                    else:
                        nc.vector.tensor_copy(o_sb[:, m2, :], psys[j][:, :tn])
                q.dma_start(
                    yt[ti, :, m2s[0] * tn : (m2s[-1] + 1) * tn],
                    o_sb[:, m2s[0] : m2s[-1] + 1, :],
                )

    nc.compile()
    return nc


def _get_nc(counts):
    key = tuple(counts)
    if key not in _cache:
        _cache[key] = _build(counts)
    return _cache[key]


def _route(x, router_w, router_b):
    """Replicate the reference router bit-for-bit (same jnp ops, same backend)."""
    import jax
    import jax.numpy as jnp

    logits = jnp.einsum("btd,ed->bte", x, router_w) + router_b
    topk_val, topk_idx = jax.lax.top_k(logits, K)
    weights = jax.nn.softmax(topk_val, axis=-1)
    return np.asarray(topk_idx), np.asarray(weights)


def kernel(x, router_w, router_b, W1, b1, W2, b2):
    from concourse.bass_utils import run_bass_kernel_spmd

    x = np.asarray(x, dtype=np.float32)
    router_w = np.asarray(router_w, dtype=np.float32)
    router_b = np.asarray(router_b, dtype=np.float32)
    W1 = np.asarray(W1, dtype=np.float32)
    b1 = np.asarray(b1, dtype=np.float32)
    W2 = np.asarray(W2, dtype=np.float32)
    b2 = np.asarray(b2, dtype=np.float32)

    B, T, _ = x.shape
    NTOK = B * T
    x_flat = x.reshape(NTOK, DIM)

    topk_idx, topk_w = _route(x, router_w, router_b)
    topk_idx = topk_idx.reshape(NTOK, K)
    topk_w = topk_w.reshape(NTOK, K).astype(np.float32)

    # Per-expert token lists + combine weights
    idx_list, w_list = [], []
    for e in range(E):
        rows, cols = np.nonzero(topk_idx == e)
        idx_list.append(rows.astype(np.int64))
        w_list.append(topk_w[rows, cols])
    counts = [len(i) for i in idx_list]

    nc = _get_nc(counts)
    tiles = _tile_list(counts)
    ntiles = len(tiles)

    bf16 = ml_dtypes.bfloat16

    # Shared token dispatch: one tile-major array used by every core.
    # Each tile's cols are packed compactly ([P, KO1*tn] used) so device
    # x loads are single contiguous transfers.
    xt = np.zeros((ntiles, P, KO1 * TN), bf16)
    tpos = [0] * E
    for ti, (e, tn) in enumerate(tiles):
        rows = x_flat[idx_list[e][tpos[e] : tpos[e] + tn]]  # [tn, DIM]
        tpos[e] += tn
        # [j, ko*P+p] -> [p, ko*tn+j]
        blk = rows.T.reshape(KO1, P, tn).transpose(1, 0, 2)  # [P, KO1, tn]
        xt[ti][:, : KO1 * tn] = blk.reshape(P, KO1 * tn).astype(bf16)

    in_maps = []
    for c in range(E):
        cols = np.r_[SH * c : SH * (c + 1), HID + SH * c : HID + SH * (c + 1)]
        w1c = np.zeros((E, P, KO1 * SW), bf16)
        w2c = np.zeros((E, P, KO2 * DIM), bf16)
        b1c = np.zeros((P, E * MO1), np.float32)
        for e in range(E):
            w1s = W1[e][:, cols]  # [DIM, SW]
            w1c[e] = (
                w1s.reshape(KO1, P, SW).transpose(1, 0, 2).reshape(P, KO1 * SW)
            ).astype(bf16)
            w2s = W2[e][SH * c : SH * (c + 1)]  # [SH, DIM]
            w2c[e] = (
                w2s.reshape(KO2, P, DIM).transpose(1, 0, 2).reshape(P, KO2 * DIM)
            ).astype(bf16)
            b1c[:, e * MO1 : (e + 1) * MO1] = b1[e][cols].reshape(MO1, P).T
        in_maps.append({"xt": xt, "w1": w1c, "b1": b1c, "w2": w2c})

    res = run_bass_kernel_spmd(nc, in_maps, core_ids=list(range(E)), **TRACE_OPTS)
    global LAST_RESULTS
    LAST_RESULTS = res

    # Sum the 8 shard partials, then combine per expert.
    y_sum = res.results[0]["yt"].astype(np.float32)
    for c in range(1, E):
        y_sum += res.results[c]["yt"]

    out_flat = np.zeros((NTOK, DIM), np.float32)
    tpos = [0] * E
    for ti, (e, tn) in enumerate(tiles):
        idx = idx_list[e][tpos[e] : tpos[e] + tn]
        w = w_list[e][tpos[e] : tpos[e] + tn]
        tpos[e] += tn
        # compact [p, m2*tn+j] -> [tn, DIM] (feature = m2*P + p)
        y = y_sum[ti][:, : MO2 * tn].reshape(P, MO2, tn).transpose(2, 1, 0)
        y = y.reshape(tn, DIM) + b2[e]
        out_flat[idx] += w[:, None] * y
    return out_flat.reshape(B, T, DIM)

